# revision 1
# baseline (speedup 1.0000x reference)
"""Contrastive loss kernel for trn2 (8 NeuronCores, SPMD).

Computes (see reference): segment-mean embeddings from f1/csv_ids and
f2/wiki_ids, logits = csv_emb @ wiki_emb.T / T, masked log-softmax losses
along both axes, returns A0*axis0 + A1*axis1.

Strategy (data-parallel over N, replicated [C,D]/[W,D] tables via AllReduce):
  - Host computes (index-only): per-core block argsort of ids, padded
    window-tiling (64 windows of 128 ids, fixed tiles/window), gather index
    lists, within-window relative ids, count-derived reciprocals.
  - Device per core: dma_gather of its f-block rows in sorted order,
    one-hot (iota==rel) matmul accumulation into PSUM per window -> partial
    [C,D]/[W,D] sum tables -> AllReduce -> divide by counts -> bf16 tables.
  - Pair terms: S1[c] = (csv_emb[c] . M1[c])/T with M1 = sum over pairs of
    wiki_emb[w_j] binned by c (same sorted order, same one-hot tiles); the
    loss is linear in the per-core partial M1, so each core reduces its own
    partial contribution to a scalar (no M1 AllReduce).
  - Logits: each core computes its C/8-row strip in bf16, exp(10*x - 40)
    (constant shift; exact for this fp32 range), ACT row-sum accumulate for
    the row lse, ones-vector matmul for column sums, one tiny AllReduce for
    the global column sums.
  - Host combines 4 scalars per core into the final loss.
"""
import sys
sys.path.insert(0, "/opt/trn_rl_repo")

import numpy as np
import ml_dtypes
from contextlib import ExitStack

import concourse.bass as bass
import concourse.tile as tile
from concourse import bacc, mybir
from concourse.bass_utils import run_bass_kernel_spmd

F32 = mybir.dt.float32
BF16 = mybir.dt.bfloat16
I16 = mybir.dt.int16
I32 = mybir.dt.int32
AF = mybir.ActivationFunctionType
OP = mybir.AluOpType

NCORES = 8
N, D = 131072, 128
C = W = 8192
B = N // NCORES            # rows per core
NWIN = 64                  # windows of 128 ids
STRIP = C // NCORES        # logits rows per core
TEMP_INV = 10.0            # 1/temperature
SHIFT = 40.0               # constant log-softmax shift
A0 = A1 = 0.5
PAD_REL = 999.0


# ----------------------------------------------------------------- device ---

LN2 = 0.6931471805599453


def _safe_ln(nc, pool, x, k, tagp):
    """ln(x) for f32 x in (0, inf) with huge dynamic range: split exponent and
    mantissa (ACT Ln LUT only covers a narrow domain). x: [128, k] SBUF tile,
    overwritten with ln(x)."""
    F32 = mybir.dt.float32
    I32 = mybir.dt.int32
    xi = x[:].bitcast(I32)
    e_i = pool.tile([128, k], I32, name=f"sl_ei_{tagp}", tag=f"sl_ei_{tagp}")
    nc.vector.tensor_scalar(out=e_i[:], in0=xi, scalar1=23, scalar2=None,
                            op0=mybir.AluOpType.logical_shift_right)
    e_f = pool.tile([128, k], F32, name=f"sl_ef_{tagp}", tag=f"sl_ef_{tagp}")
    nc.vector.tensor_copy(out=e_f[:], in_=e_i[:])
    m_i = pool.tile([128, k], I32, name=f"sl_mi_{tagp}", tag=f"sl_mi_{tagp}")
    nc.vector.tensor_scalar(out=m_i[:], in0=xi, scalar1=0x007FFFFF,
                            scalar2=0x3F800000,
                            op0=mybir.AluOpType.bitwise_and,
                            op1=mybir.AluOpType.bitwise_or)
    ln_m = pool.tile([128, k], F32, name=f"sl_lm_{tagp}", tag=f"sl_lm_{tagp}")
    nc.scalar.activation(out=ln_m[:], in_=m_i[:].bitcast(F32),
                         func=mybir.ActivationFunctionType.Ln)
    # e' = e*ln2 - 127*ln2
    nc.vector.tensor_scalar(out=e_f[:], in0=e_f[:], scalar1=LN2,
                            scalar2=127.0 * LN2, op0=mybir.AluOpType.mult,
                            op1=mybir.AluOpType.subtract)
    nc.vector.tensor_tensor(out=x[:], in0=ln_m[:], in1=e_f[:],
                            op=mybir.AluOpType.add)


def _build(nt_c, nt_w, stop="FULL"):
    """Build the SPMD Bass program. nt_c/nt_w: total 128-row tiles per side.
    stop: debug knob to truncate the program after a phase."""
    nc = bacc.Bacc("TRN2", target_bir_lowering=False, debug=False,
                   num_devices=NCORES, num_swdge_queues=4)

    f1b = nc.dram_tensor("f1b", [128, nt_c * D], BF16, kind="ExternalInput")
    f2b = nc.dram_tensor("f2b", [128, nt_w * D], BF16, kind="ExternalInput")
    rel_c = nc.dram_tensor("rel_c", [128, nt_c], F32, kind="ExternalInput")
    rel_w = nc.dram_tensor("rel_w", [128, nt_w], F32, kind="ExternalInput")
    pg_c = nc.dram_tensor("pg_c", [128, B // 16], I16, kind="ExternalInput")
    pg_w = nc.dram_tensor("pg_w", [128, B // 16], I16, kind="ExternalInput")
    sidx = nc.dram_tensor("sidx", [128, STRIP // 16], I16, kind="ExternalInput")
    rwin_c = nc.dram_tensor("rwin_c", [128, NWIN], F32, kind="ExternalInput")
    rwin_w = nc.dram_tensor("rwin_w", [128, NWIN], F32, kind="ExternalInput")
    g1s = nc.dram_tensor("g1s", [128, STRIP // 128], F32, kind="ExternalInput")
    r1c = nc.dram_tensor("r1c", [128, B // 128], F32, kind="ExternalInput")
    r0w = nc.dram_tensor("r0w", [128, B // 128], F32, kind="ExternalInput")
    g0f = nc.dram_tensor("g0f", [128, W // 128], F32, kind="ExternalInput")
    osc = nc.dram_tensor("osc", [1, 8], F32, kind="ExternalOutput")

    with tile.TileContext(nc) as tc, ExitStack() as ctx:
        const = ctx.enter_context(tc.tile_pool(name="const", bufs=1))
        persist = ctx.enter_context(tc.tile_pool(name="persist", bufs=1))
        dram = ctx.enter_context(tc.tile_pool(name="dram", bufs=1, space="DRAM"))

        # ---- constants
        iota_i = const.tile([128, 128], I32)
        nc.gpsimd.iota(iota_i[:], pattern=[[1, 128]], base=0, channel_multiplier=0)
        iota_bf = const.tile([128, 128], BF16)
        nc.vector.tensor_copy(out=iota_bf[:], in_=iota_i[:])
        pid_i = const.tile([128, 1], I32)
        nc.gpsimd.iota(pid_i[:], pattern=[[1, 1]], base=0, channel_multiplier=1)
        pid_f = const.tile([128, 1], F32)
        nc.vector.tensor_copy(out=pid_f[:], in_=pid_i[:])
        ident = const.tile([128, 128], BF16)
        nc.vector.tensor_scalar(out=ident[:], in0=iota_bf[:], scalar1=pid_f[:, 0:1],
                                scalar2=None, op0=OP.is_equal)
        ones_bf = const.tile([128, 1], BF16)
        nc.vector.memset(ones_bf[:], 1.0)
        bias_m40 = const.tile([128, 1], F32)
        nc.vector.memset(bias_m40[:], -SHIFT)

        # rel / rwin / g inputs
        rel_t = {}
        rel_t["c"] = const.tile([128, nt_c], F32, name="rel_c_t", tag="rel_c")
        nc.sync.dma_start(out=rel_t["c"][:], in_=rel_c[:, :])
        rel_t["w"] = const.tile([128, nt_w], F32, name="rel_w_t", tag="rel_w")
        nc.sync.dma_start(out=rel_t["w"][:], in_=rel_w[:, :])
        rwin_t = {}
        rwin_t["c"] = const.tile([128, NWIN], F32, name="rwin_c_t", tag="rwin_c")
        nc.sync.dma_start(out=rwin_t["c"][:], in_=rwin_c[:, :])
        rwin_t["w"] = const.tile([128, NWIN], F32, name="rwin_w_t", tag="rwin_w")
        nc.sync.dma_start(out=rwin_t["w"][:], in_=rwin_w[:, :])

        pgidx_t = {}
        for key, src in (("c", pg_c), ("w", pg_w)):
            t = const.tile([128, B // 16], I16, name=f"pgidx_{key}_t", tag=f"pgidx_{key}")
            nc.sync.dma_start(out=t[:], in_=src[:, :])
            pgidx_t[key] = t

        # ---- DRAM scratch
        sum_dram = {"c": dram.tile([128, C], F32, name="sum_dram_c"),
                    "w": dram.tile([128, C], F32, name="sum_dram_w")}
        sum_ar = {"c": dram.tile([128, C], F32, name="sum_ar_c"),
                  "w": dram.tile([128, C], F32, name="sum_ar_w")}
        emb_dram = {"c": dram.tile([C, D], BF16, name="emb_c_dram"), "w": dram.tile([W, D], BF16, name="emb_w_dram")}
        cs_dram = dram.tile([1, W], F32)
        cs_ar = dram.tile([1, W], F32)

        sides = [("c", f1b, nt_c), ("w", f2b, nt_w)]

        # ================= phase A: segment sums =================
        with tc.tile_pool(name="ga", bufs=2) as gpool, \
             tc.tile_pool(name="wka", bufs=12) as wk, \
             tc.tile_pool(name="psa", bufs=7, space="PSUM") as psa:
            for side, fparam, nt in sides:
                tpw = nt // NWIN
                ch = 8         # tiles per gather chunk (1024 idxs: fits ring)
                off = 0 if side == "c" else C
                grp_ps = None
                fall = gpool.tile([128, nt, D], BF16, tag="fall")
                nc.sync.dma_start(out=fall[:], in_=fparam[:, :])
                for g in range(nt // ch):
                    gath = fall[:, g * ch:(g + 1) * ch, :]
                    for tl in range(ch):
                        t = g * ch + tl
                        w = t // tpw
                        s = t % tpw
                        grp, k = w // 4, w % 4
                        oh = wk.tile([128, 128], BF16, tag="oh")
                        nc.vector.tensor_scalar(
                            out=oh[:], in0=iota_bf[:], scalar1=rel_t[side][:, t:t + 1],
                            scalar2=None, op0=OP.is_equal)
                        if s == 0 and k == 0:
                            grp_ps = psa.tile([128, 512], F32, tag="winps")
                        nc.tensor.matmul(grp_ps[:, k * 128:(k + 1) * 128],
                                         lhsT=oh[:], rhs=gath[:, tl, :],
                                         start=(s == 0), stop=(s == tpw - 1))
                        if s == tpw - 1 and k == 3:
                            stg = wk.tile([128, 512], F32, tag="stg")
                            eng = nc.vector if grp % 2 == 0 else nc.scalar
                            if grp % 2 == 0:
                                nc.vector.tensor_copy(out=stg[:], in_=grp_ps[:])
                            else:
                                nc.scalar.copy(out=stg[:], in_=grp_ps[:])
                            nc.sync.dma_start(
                                out=sum_dram[side][:, grp * 512:(grp + 1) * 512],
                                in_=stg[:])
                if stop not in ("A",):
                    nc.gpsimd.collective_compute(
                        "AllReduce", OP.add,
                        replica_groups=[list(range(NCORES))],
                        ins=[sum_dram[side].opt()], outs=[sum_ar[side].opt()])

        done = stop in ("A", "AR")
        if done:
            out_sc = persist.tile([1, 8], F32, tag="osc_dbg")
            nc.vector.memset(out_sc[:], 2.0)
            nc.sync.dma_start(out=osc[:, :], in_=out_sc[:])

        # ================= divide by counts, make bf16 tables =================
        embp_cm = tc.tile_pool(name="embp", bufs=1)
        embp = embp_cm.__enter__()
        emb_cd = {}
        for side, _, _ in (sides if not done else []):
            ecd = embp.tile([128, NWIN, 128], F32, name=f"ecd_{side}", tag="ecd", bufs=2)
            nc.sync.dma_start(out=ecd[:], in_=sum_ar[side][:])
            ebb = embp.tile([128, NWIN, 128], BF16, name=f"ebb_{side}", tag="ebb", bufs=2)
            for w in range(NWIN):
                eng = nc.vector if w % 2 == 0 else nc.scalar
                if w % 2 == 0:
                    nc.vector.tensor_scalar(
                        out=ebb[:, w, :], in0=ecd[:, w, :],
                        scalar1=rwin_t[side][:, w:w + 1], scalar2=None, op0=OP.mult)
                else:
                    nc.scalar.mul(ebb[:, w, :], ecd[:, w, :],
                                  rwin_t[side][:, w:w + 1])
            nc.sync.dma_start(
                out=emb_dram[side].rearrange("(w p) d -> p w d", p=128), in_=ebb[:])

        if stop == "DIV" and not done:
            done = True
            out_sc = persist.tile([1, 8], F32, tag="osc_dbg")
            nc.vector.memset(out_sc[:], 3.0)
            nc.sync.dma_start(out=osc[:, :], in_=out_sc[:])

        embp_cm.__exit__(None, None, None)

        if stop == "B" and not done:
            done = True
            out_sc = persist.tile([1, 8], F32, tag="osc_dbg")
            nc.vector.memset(out_sc[:], 4.0)
            nc.sync.dma_start(out=osc[:, :], in_=out_sc[:])

        # ================= logits strip =================
        # csv lhsT: gather this core's 1024 csv_emb rows, PE-transpose to [d, c]
        sidx_t = const.tile([128, STRIP // 16], I16)
        nc.sync.dma_start(out=sidx_t[:], in_=sidx[:, :])
        csv_lhsT = persist.tile([128, STRIP], BF16, tag="csv_lhsT")
        wiki_T = persist.tile([128, W], BF16, tag="wiki_T")
        if not done:
            nc.sync.dma_start_transpose(out=wiki_T[:], in_=emb_dram["w"][:, :])

        rs_all = persist.tile([128, 8 * 16], F32, tag="rs_all")
        colsum = persist.tile([1, W], F32, tag="colsum")

        with tc.tile_pool(name="wkl", bufs=3) as wk, \
             tc.tile_pool(name="psl", bufs=2, space="PSUM") as psl, \
             tc.tile_pool(name="pst", bufs=2, space="PSUM") as pst:
            sgath = wk.tile([128, 8, D], BF16, tag="sgath")
            if not done:
                nc.gpsimd.dma_gather(
                    out_ap=sgath[:], in_ap=emb_dram["c"][:, :], idxs_ap=sidx_t[:],
                    num_idxs=STRIP, num_idxs_reg=STRIP, elem_size=D)
            for t in range(8 if not done else 0):
                tp = pst.tile([128, 128], BF16, tag="trps")
                nc.tensor.transpose(tp[:], sgath[:, t, :], ident[:])
                nc.scalar.copy(out=csv_lhsT[:, t * 128:(t + 1) * 128], in_=tp[:])

            for chunk in range(16 if not done else 0):
                cs_ps = pst.tile([1, 512], F32, tag="csps")
                for sub in range(8):
                    lp = psl.tile([128, 512], F32, tag="lps")
                    nc.tensor.matmul(
                        lp[:], lhsT=csv_lhsT[:, sub * 128:(sub + 1) * 128],
                        rhs=wiki_T[:, chunk * 512:(chunk + 1) * 512],
                        start=True, stop=True)
                    ex = wk.tile([128, 512], BF16, tag="ex")
                    col = sub * 16 + chunk
                    nc.scalar.activation(
                        out=ex[:], in_=lp[:], func=AF.Exp, scale=TEMP_INV,
                        bias=bias_m40[:, 0:1])
                    nc.vector.tensor_reduce(
                        out=rs_all[:, col:col + 1], in_=ex[:],
                        axis=mybir.AxisListType.XYZW, op=OP.add)
                    nc.tensor.matmul(cs_ps[:], lhsT=ones_bf[:], rhs=ex[:],
                                     start=(sub == 0), stop=(sub == 7))
                nc.vector.tensor_copy(
                    out=colsum[0:1, chunk * 512:(chunk + 1) * 512], in_=cs_ps[:])

        if stop == "LOG" and not done:
            done = True
            out_sc = persist.tile([1, 8], F32, tag="osc_dbg")
            nc.vector.memset(out_sc[:], 5.0)
            nc.sync.dma_start(out=osc[:, :], in_=out_sc[:])

        # ============ phase B: pair-dot terms ============
        # u1 = sum_n r1[c_n] * (csv_emb[c_n] . wiki_emb[w_n]); u0 with r0[w_n].
        # Gather both embedding rows per pair in natural order; dots on DVE.
        u_acc = {}
        with tc.tile_pool(name="gb", bufs=8) as gpool, \
             tc.tile_pool(name="wkb", bufs=4) as wk:
            nb = B // 128 // 16          # tiles per gather chunk (16 chunks)
            rr_t = {}
            for key, src in (("c", r1c), ("w", r0w)):
                t = wk.tile([128, B // 128], F32, name=f"rr_{key}", tag=f"rr_{key}")
                nc.sync.dma_start(out=t[:], in_=src[:, :])
                rr_t[key] = t
            uac = persist.tile([128, 2 * 16], F32, name="uac", tag="uac")
            u_acc["c"] = uac[:, 0:16]
            u_acc["w"] = uac[:, 16:32]
            for g in range(16):
                gc = gpool.tile([128, nb, D], BF16, tag="gbc")
                nc.gpsimd.dma_gather(
                    out_ap=gc[:], in_ap=emb_dram["c"][:, :],
                    idxs_ap=pgidx_t["c"][:, g * nb * 8:(g + 1) * nb * 8],
                    num_idxs=nb * 128, num_idxs_reg=nb * 128, elem_size=D,
                    single_packet=False, queue_num=(2 * g) % 4)
                gw = gpool.tile([128, nb, D], BF16, tag="gbw")
                nc.gpsimd.dma_gather(
                    out_ap=gw[:], in_ap=emb_dram["w"][:, :],
                    idxs_ap=pgidx_t["w"][:, g * nb * 8:(g + 1) * nb * 8],
                    num_idxs=nb * 128, num_idxs_reg=nb * 128, elem_size=D,
                    single_packet=False, queue_num=(2 * g + 1) % 4)
                prod = wk.tile([128, nb, D], BF16, tag="prod")
                nc.vector.tensor_tensor(out=prod[:], in0=gc[:], in1=gw[:],
                                        op=OP.mult)
                dots = wk.tile([128, nb], F32, tag="dots")
                nc.vector.tensor_reduce(out=dots[:], in_=prod[:],
                                        axis=mybir.AxisListType.X, op=OP.add)
                sc1 = wk.tile([128, nb], F32, tag="sc1")
                nc.vector.scalar_tensor_tensor(
                    out=sc1[:], in0=dots[:], scalar=1.0, op0=OP.mult,
                    in1=rr_t["c"][:, g * nb:(g + 1) * nb], op1=OP.mult,
                    accum_out=u_acc["c"][:, g:g + 1])
                sc0 = wk.tile([128, nb], F32, tag="sc0")
                nc.vector.scalar_tensor_tensor(
                    out=sc0[:], in0=dots[:], scalar=1.0, op0=OP.mult,
                    in1=rr_t["w"][:, g * nb:(g + 1) * nb], op1=OP.mult,
                    accum_out=u_acc["w"][:, g:g + 1])


        # column-sum AllReduce (global over all C rows)
        if not done:
            nc.sync.dma_start(out=cs_dram[:], in_=colsum[:])
            nc.gpsimd.collective_compute(
                "AllReduce", OP.add, replica_groups=[list(range(NCORES))],
                ins=[cs_dram.opt()], outs=[cs_ar.opt()])

        if not done:
            # ================= final scalars =================
            fin = ctx.enter_context(tc.tile_pool(name="fin", bufs=1))
            g1s_t = fin.tile([128, STRIP // 128], F32)
            nc.sync.dma_start(out=g1s_t[:], in_=g1s[:, :])
            g0f_t = fin.tile([128, W // 128], F32)
            nc.sync.dma_start(out=g0f_t[:], in_=g0f[:, :])
            cs_t = fin.tile([128, W // 128], F32)
            nc.sync.dma_start(
                out=cs_t[:], in_=cs_ar.rearrange("a (j p) -> p (a j)", p=128))

            # u1/u0: sum over windows then partitions
            out_sc = fin.tile([1, 8], F32)
            nc.vector.memset(out_sc[:], 0.0)
            for j, side in enumerate(("c", "w")):
                red = fin.tile([128, 1], F32, name=f"red_{side}", tag=f"red_{side}")
                nc.vector.tensor_reduce(out=red[:], in_=u_acc[side],
                                        axis=mybir.AxisListType.XYZW, op=OP.add)
                sc = fin.tile([1, 1], F32, name=f"sc_{side}", tag=f"sc_{side}")
                nc.gpsimd.tensor_reduce(out=sc[:], in_=red[:],
                                        axis=mybir.AxisListType.C, op=OP.add)
                nc.vector.tensor_copy(out=out_sc[0:1, j:j + 1], in_=sc[:])

            # v1: strip row lse
            rsum = fin.tile([128, 8], F32)
            for sub in range(8):
                nc.vector.tensor_reduce(
                    out=rsum[:, sub:sub + 1], in_=rs_all[:, sub * 16:(sub + 1) * 16],
                    axis=mybir.AxisListType.XYZW, op=OP.add)
            _safe_ln(nc, fin, rsum, 8, "r")
            lse1 = rsum
            scr1 = fin.tile([128, 8], F32)
            v1p = fin.tile([128, 1], F32)
            nc.vector.scalar_tensor_tensor(
                out=scr1[:], in0=lse1[:], scalar=1.0, op0=OP.mult,
                in1=g1s_t[:], op1=OP.mult, accum_out=v1p[:])
            v1 = fin.tile([1, 1], F32)
            nc.gpsimd.tensor_reduce(out=v1[:], in_=v1p[:],
                                    axis=mybir.AxisListType.C, op=OP.add)
            nc.vector.tensor_copy(out=out_sc[0:1, 2:3], in_=v1[:])

            # v0: global col lse (same on every core)
            _safe_ln(nc, fin, cs_t, W // 128, "c")
            scr0 = fin.tile([128, W // 128], F32)
            v0p = fin.tile([128, 1], F32)
            nc.vector.scalar_tensor_tensor(
                out=scr0[:], in0=cs_t[:], scalar=1.0, op0=OP.mult,
                in1=g0f_t[:], op1=OP.mult, accum_out=v0p[:])
            v0 = fin.tile([1, 1], F32)
            nc.gpsimd.tensor_reduce(out=v0[:], in_=v0p[:],
                                    axis=mybir.AxisListType.C, op=OP.add)
            nc.vector.tensor_copy(out=out_sc[0:1, 3:4], in_=v0[:])

            nc.sync.dma_start(out=osc[:, :], in_=out_sc[:])

    nc.finalize()
    return nc


# ------------------------------------------------------------------- host ---

def _wrap16(a):
    """[num] int16 -> [128, num//16] gather-index layout (16-wrap, 8x repl)."""
    return np.ascontiguousarray(np.tile(a.reshape(-1, 16).T, (8, 1)))


def _side_prep(ids, tpw):
    """Padded window tiling for one sorted side of one core's block."""
    nt = NWIN * tpw
    perm = np.argsort(ids, kind="stable")
    srt = ids[perm]
    gi = np.zeros(nt * 128, np.int64)
    rel = np.full(nt * 128, PAD_REL, np.float32)
    starts = np.searchsorted(srt, np.arange(NWIN) * 128)
    ends = np.searchsorted(srt, np.arange(1, NWIN + 1) * 128)
    for w in range(NWIN):
        s, e = starts[w], ends[w]
        base = w * tpw * 128
        cnt = e - s
        gi[base:base + cnt] = perm[s:e]
        rel[base:base + cnt] = (srt[s:e] - w * 128).astype(np.float32)
    return (
        gi, (rel != PAD_REL),
        np.ascontiguousarray(rel.reshape(nt, 128).T),
    )


_CACHE = {}


def _run(inputs, trace=False, tmpdir=None):
    f1 = np.asarray(inputs["f1"], np.float32)
    f2 = np.asarray(inputs["f2"], np.float32)
    ci = np.asarray(inputs["csv_ids"]).astype(np.int64)
    wi = np.asarray(inputs["wiki_ids"]).astype(np.int64)

    cnt_c = np.bincount(ci, minlength=C).astype(np.float32)
    cnt_w = np.bincount(wi, minlength=W).astype(np.float32)
    r_c = 1.0 / np.maximum(cnt_c, 1.0)
    r_w = 1.0 / np.maximum(cnt_w, 1.0)
    g_c = (cnt_c > 0).astype(np.float32)
    g_w = (cnt_w > 0).astype(np.float32)

    # fixed tiles/window across all cores & sides (one compiled program)
    tpw = 1
    for ids in (ci, wi):
        for i in range(NCORES):
            blk = ids[i * B:(i + 1) * B]
            h = np.bincount(blk >> 7, minlength=NWIN).max()
            tpw = max(tpw, int(-(-h // 128)))
    nt = NWIN * tpw

    import os as _os
    stop = _os.environ.get("KSTOP", "FULL")
    key = (nt, nt, stop)
    if key not in _CACHE:
        _CACHE[key] = _build(nt, nt, stop=stop)
    nc = _CACHE[key]

    rwin_c_arr = np.ascontiguousarray(r_c.reshape(NWIN, 128).T)
    rwin_w_arr = np.ascontiguousarray(r_w.reshape(NWIN, 128).T)
    g0_arr = np.ascontiguousarray(g_w.reshape(W // 128, 128).T)

    in_maps = []
    for i in range(NCORES):
        sl = slice(i * B, (i + 1) * B)
        gi_c, valid_c, rel_c_a = _side_prep(ci[sl], tpw)
        gi_w, valid_w, rel_w_a = _side_prep(wi[sl], tpw)
        nt = NWIN * tpw
        f1p = f1[sl].astype(ml_dtypes.bfloat16)[gi_c]
        f1p[~valid_c] = 0
        f1p = np.ascontiguousarray(
            f1p.reshape(nt, 128, D).transpose(1, 0, 2).reshape(128, nt * D))
        f2p = f2[sl].astype(ml_dtypes.bfloat16)[gi_w]
        f2p[~valid_w] = 0
        f2p = np.ascontiguousarray(
            f2p.reshape(nt, 128, D).transpose(1, 0, 2).reshape(128, nt * D))
        cib, wib = ci[sl], wi[sl]
        pg_c_a = _wrap16(cib.astype(np.int16))
        pg_w_a = _wrap16(wib.astype(np.int16))
        r1c_arr = np.ascontiguousarray(r_c[cib].reshape(B // 128, 128).T)
        r0w_arr = np.ascontiguousarray(r_w[wib].reshape(B // 128, 128).T)
        strip = np.arange(i * STRIP, (i + 1) * STRIP, dtype=np.int16)
        g1s_arr = np.ascontiguousarray(
            g_c[i * STRIP:(i + 1) * STRIP].reshape(STRIP // 128, 128).T)
        in_maps.append({
            "f1b": f1p,
            "f2b": f2p,
            "rel_c": rel_c_a, "rel_w": rel_w_a,
            "pg_c": pg_c_a, "pg_w": pg_w_a,
            "sidx": _wrap16(strip),
            "rwin_c": rwin_c_arr, "rwin_w": rwin_w_arr,
            "g1s": g1s_arr, "g0f": g0_arr,
            "r1c": r1c_arr, "r0w": r0w_arr,
        })

    res = run_bass_kernel_spmd(nc, in_maps, core_ids=list(range(NCORES)),
                               trace=trace, tmpdir=tmpdir)

    # combine: per core osc = [u1, u0, v1', v0']
    sc = np.stack([res.results[i]["osc"][0] for i in range(NCORES)])
    u1 = float(sc[:, 0].sum())
    u0 = float(sc[:, 1].sum())
    v1 = float(sc[:, 2].sum())          # per-strip partials
    v0 = float(sc[0, 3])                # identical on every core
    G1 = float(g_c.sum())
    G0 = float(g_w.sum())
    ax1 = -(TEMP_INV * u1 - (v1 + SHIFT * G1)) / C
    ax0 = -(TEMP_INV * u0 - (v0 + SHIFT * G0)) / W
    loss = A0 * ax0 + A1 * ax1
    return np.float32(loss), res


def kernel(**inputs) -> np.ndarray:
    out, _ = _run(inputs)
    return out



# revision 20
# speedup vs baseline: 1.8179x; 1.8179x over previous
"""Contrastive loss kernel for trn2 (8 NeuronCores, SPMD) — v2.

Reference math: segment-mean embeddings from f1/csv_ids and f2/wiki_ids,
logits = csv_emb @ wiki_emb.T / T, masked log-softmax losses along both
axes over unique observed pairs, loss = A0*axis0 + A1*axis1.

Strategy (strip sharding, one small AllGather, no AllReduce of tables):
  - Core i OWNS id strip [1024*i, 1024*(i+1)) on both sides. Host routes
    every f1 row to the core owning its csv id (and f2 rows by wiki id),
    sorted by id, tiled into 8 windows of 128 ids per strip. One-hot
    scatter matrices are PRECOMPUTED ON HOST with the segment-mean
    reciprocal folded into the values, so a chain of 128x128 matmuls per
    window directly yields the strip of the mean table. No collective
    reduce needed.
  - Only the wiki strip tables are AllGathered (bf16, 2MB total) into a
    replicated [W, D] table: logits need all wiki rows, but csv rows are
    only needed for the local strip.
  - Logits: each core computes its strip rows x full W in 16 chunks of
    512 columns; exp(10*x - 40) on ACT with accum_out giving row sums
    for free; column sums via DVE adds + one ones-matmul per chunk.
  - Pair terms: loss needs u = sum over unique pairs of
    dot(csv_emb[c], wiki_emb[w]) * (1/ru[c] + 1/cu[w]). Pairs are routed
    by csv strip; wiki rows are dma_gathered (from the AllGathered
    table) in csv-sorted order and segment-summed into K[c] via one-hot
    matmuls whose values are the pair weights; u_partial = sum over the
    strip of csv_emb . K on DVE. Gathers overlap the logits phase.
  - Host combines per-core scalars into the final loss.
"""
import sys
sys.path.insert(0, "/opt/trn_rl_repo")

import numpy as np
import ml_dtypes
from contextlib import ExitStack

import concourse.bass as bass
import concourse.tile as tile
from concourse import bacc, mybir
from concourse.bass_utils import run_bass_kernel_spmd

F32 = mybir.dt.float32
BF16 = mybir.dt.bfloat16
I16 = mybir.dt.int16
I32 = mybir.dt.int32
AF = mybir.ActivationFunctionType
OP = mybir.AluOpType

NCORES = 8
N, D = 131072, 128
C = W = 8192
STRIP = C // NCORES        # ids per strip (1024)
NWIN = STRIP // 128        # windows per strip (8)
TEMP_INV = 10.0            # 1/temperature
SHIFT = 40.0               # constant log-softmax shift
A0 = A1 = 0.5

LN2 = 0.6931471805599453


def _safe_ln(nc, pool, x, k, tagp):
    """ln(x) for f32 x in (0, inf) with huge dynamic range: split exponent and
    mantissa (ACT Ln LUT only covers a narrow domain). x: [128, k] SBUF tile,
    overwritten with ln(x)."""
    xi = x[:].bitcast(I32)
    e_i = pool.tile([128, k], I32, name=f"sl_ei_{tagp}", tag=f"sl_ei_{tagp}")
    nc.vector.tensor_scalar(out=e_i[:], in0=xi, scalar1=23, scalar2=None,
                            op0=OP.logical_shift_right)
    e_f = pool.tile([128, k], F32, name=f"sl_ef_{tagp}", tag=f"sl_ef_{tagp}")
    nc.vector.tensor_copy(out=e_f[:], in_=e_i[:])
    m_i = pool.tile([128, k], I32, name=f"sl_mi_{tagp}", tag=f"sl_mi_{tagp}")
    nc.vector.tensor_scalar(out=m_i[:], in0=xi, scalar1=0x007FFFFF,
                            scalar2=0x3F800000,
                            op0=OP.bitwise_and,
                            op1=OP.bitwise_or)
    ln_m = pool.tile([128, k], F32, name=f"sl_lm_{tagp}", tag=f"sl_lm_{tagp}")
    nc.scalar.activation(out=ln_m[:], in_=m_i[:].bitcast(F32),
                         func=AF.Ln)
    # e' = e*ln2 - 127*ln2
    nc.vector.tensor_scalar(out=e_f[:], in0=e_f[:], scalar1=LN2,
                            scalar2=127.0 * LN2, op0=OP.mult,
                            op1=OP.subtract)
    nc.vector.tensor_tensor(out=x[:], in0=ln_m[:], in1=e_f[:],
                            op=OP.add)


# ----------------------------------------------------------------- device ---

def _build(tpwa, tpwb, stop="FULL"):
    """Build the SPMD Bass program. tpwa/tpwb: tiles per 128-id window for
    phase A (segment sums) and phase B (pair terms). stop: debug knob —
    NOAG (phase A only, no collective), A (phase A + AllGather), LOG
    (through logits, no gathers/pair terms), FULL."""
    nc = bacc.Bacc("TRN2", target_bir_lowering=False, debug=False,
                   num_devices=NCORES, num_swdge_queues=4)

    nta = NWIN * tpwa          # phase A tiles per side (per core)
    ntb = NWIN * tpwb          # phase B tiles (per core)

    fw = nc.dram_tensor("fw", [128, nta * D], BF16, kind="ExternalInput")
    ow = nc.dram_tensor("ow", [128, nta * 128], BF16, kind="ExternalInput")
    fc = nc.dram_tensor("fc", [128, nta * D], BF16, kind="ExternalInput")
    oc = nc.dram_tensor("oc", [128, nta * 128], BF16, kind="ExternalInput")
    ob = nc.dram_tensor("ob", [128, ntb * 128], BF16, kind="ExternalInput")
    rwc = nc.dram_tensor("rwc", [128, NWIN], F32, kind="ExternalInput")
    rww = nc.dram_tensor("rww", [128, NWIN], F32, kind="ExternalInput")
    gidx = nc.dram_tensor("gidx", [128, ntb * 8], I16, kind="ExternalInput")
    g1s = nc.dram_tensor("g1s", [128, NWIN], F32, kind="ExternalInput")
    g0f = nc.dram_tensor("g0f", [128, W // 128], F32, kind="ExternalInput")
    osc = nc.dram_tensor("osc", [1, 8], F32, kind="ExternalOutput")

    with tile.TileContext(nc) as tc, ExitStack() as ctx:
        const = ctx.enter_context(tc.tile_pool(name="const", bufs=1))
        persist = ctx.enter_context(tc.tile_pool(name="persist", bufs=1))
        dram = ctx.enter_context(tc.tile_pool(name="dram", bufs=1, space="DRAM"))

        # ---- constants
        iota_i = const.tile([128, 128], I32)
        nc.gpsimd.iota(iota_i[:], pattern=[[1, 128]], base=0, channel_multiplier=0)
        iota_bf = const.tile([128, 128], BF16)
        nc.vector.tensor_copy(out=iota_bf[:], in_=iota_i[:])
        pid_i = const.tile([128, 1], I32)
        nc.gpsimd.iota(pid_i[:], pattern=[[1, 1]], base=0, channel_multiplier=1)
        pid_f = const.tile([128, 1], F32)
        nc.vector.tensor_copy(out=pid_f[:], in_=pid_i[:])
        ident = const.tile([128, 128], BF16)
        nc.vector.tensor_scalar(out=ident[:], in0=iota_bf[:], scalar1=pid_f[:, 0:1],
                                scalar2=None, op0=OP.is_equal)
        ones_bf = const.tile([128, 1], BF16)
        nc.vector.memset(ones_bf[:], 1.0)
        bias_m40 = const.tile([128, 1], F32)
        nc.vector.memset(bias_m40[:], -SHIFT)

        gidx_t = const.tile([128, ntb * 8], I16)
        nc.sync.dma_start(out=gidx_t[:], in_=gidx[:, :])
        rw_t = {}
        for key_, src in (("c", rwc), ("w", rww)):
            t = const.tile([128, NWIN], F32, name=f"rw_{key_}_t", tag=f"rw_{key_}")
            nc.sync.dma_start(out=t[:], in_=src[:, :])
            rw_t[key_] = t
        g1s_t = const.tile([128, NWIN], F32)
        nc.sync.dma_start(out=g1s_t[:], in_=g1s[:, :])
        g0f_t = const.tile([128, W // 128], F32)
        nc.sync.dma_start(out=g0f_t[:], in_=g0f[:, :])

        # ---- DRAM scratch
        w_strip = dram.tile([STRIP, D], BF16, name="w_strip")
        w_full = dram.tile([C, D], BF16, name="w_full", addr_space="Shared")
        cs_dram = dram.tile([1, W], F32, name="cs_dram")
        cs_ar = dram.tile([1, W], F32, name="cs_ar", addr_space="Shared")

        # ---- persistent SBUF
        cstrip = persist.tile([128, NWIN, 128], BF16, tag="cstrip")
        csv_lhsT = persist.tile([128, STRIP], BF16, tag="csv_lhsT")
        wiki_T = persist.tile([128, W], BF16, tag="wiki_T")
        rs_all = persist.tile([128, NWIN * 16], F32, tag="rs_all")
        colsum = persist.tile([1, W], F32, tag="colsum")
        u_acc = persist.tile([128, NWIN], F32, tag="u_acc")
        nc.vector.memset(u_acc[:], 0.0)
        ob_all = persist.tile([128, ntb, 128], BF16, tag="ob_all")

        # ================= phase A: strip segment-mean tables =================
        with tc.tile_pool(name="pha", bufs=3) as pha, \
             tc.tile_pool(name="psa", bufs=3, space="PSUM") as psa:
            for side, fparam, oparam in (("w", fw, ow), ("c", fc, oc)):
                if side == "w":
                    strip_sb = pha.tile([128, NWIN, 128], BF16, tag="wstrip",
                                        bufs=1, name="wstrip_sb")
                else:
                    strip_sb = cstrip
                for win in range(NWIN):
                    ft = pha.tile([128, tpwa, D], BF16, tag="ft", name="ft")
                    nc.sync.dma_start(
                        out=ft[:], in_=fparam[:, win * tpwa * D:(win + 1) * tpwa * D])
                    ot = pha.tile([128, tpwa, 128], BF16, tag="ot", name="ot")
                    nc.sync.dma_start(
                        out=ot[:], in_=oparam[:, win * tpwa * 128:(win + 1) * tpwa * 128])
                    ps = psa.tile([128, 128], F32, tag="ps", name="ps_a")
                    for t in range(tpwa):
                        nc.tensor.matmul(ps[:], lhsT=ot[:, t, :], rhs=ft[:, t, :],
                                         start=(t == 0), stop=(t == tpwa - 1))
                    if win % 2 == 0:
                        nc.vector.tensor_scalar(
                            out=strip_sb[:, win, :], in0=ps[:],
                            scalar1=rw_t[side][:, win:win + 1], scalar2=None,
                            op0=OP.mult)
                    else:
                        nc.scalar.mul(strip_sb[:, win, :], ps[:],
                                      rw_t[side][:, win:win + 1])
                if side == "w":
                    nc.sync.dma_start(
                        out=w_strip.rearrange("(w p) d -> p w d", p=128),
                        in_=strip_sb[:])
                    if stop != "NOAG":
                        nc.gpsimd.collective_compute(
                            "AllGather", OP.bypass,
                            replica_groups=[list(range(NCORES))],
                            ins=[w_strip.opt()], outs=[w_full.opt()])

        done = stop in ("NOAG", "A")
        if done:
            out_dbg = persist.tile([1, 8], F32, tag="osc_dbg")
            nc.vector.memset(out_dbg[:], 2.0)
            nc.sync.dma_start(out=osc[:, :], in_=out_dbg[:])

        # csv strip lhsT via PE transposes (overlaps the AllGather)
        with tc.tile_pool(name="pst", bufs=2, space="PSUM") as pst:
            for s in range(NWIN if not done else 0):
                tp = pst.tile([128, 128], BF16, tag="tp", name="tp")
                nc.tensor.transpose(tp[:], cstrip[:, s, :], ident[:])
                nc.scalar.copy(out=csv_lhsT[:, s * 128:(s + 1) * 128], in_=tp[:])

        if not done:
            # full wiki table transposed for logits rhs (waits on AllGather)
            nc.sync.dma_start_transpose(out=wiki_T[:], in_=w_full[:, :])
            # pair one-hots (values carry the 1/ru + 1/cu pair weights)
            nc.sync.dma_start(out=ob_all[:], in_=ob[:, :])

        # ============ logits strip + pair terms (interleaved) ============
        with tc.tile_pool(name="gbp", bufs=NWIN) as gbp, \
             tc.tile_pool(name="wkl", bufs=6) as wk, \
             tc.tile_pool(name="psl", bufs=3, space="PSUM") as psl, \
             tc.tile_pool(name="psc", bufs=2, space="PSUM") as psc, \
             tc.tile_pool(name="psb", bufs=2, space="PSUM") as psb:
            # wiki-row gathers (gpsimd stream: right after the AllGather);
            # split into <=1024-index calls (SWDGE descriptor-ring capacity)
            do_b = (stop == "FULL") and not done
            gw_tiles = []
            for win in range(NWIN if do_b else 0):
                gwt = gbp.tile([128, tpwb, 128], BF16, tag="gw", name="gw")
                for t0 in range(0, tpwb, 8):
                    t1 = min(t0 + 8, tpwb)
                    nt_ = t1 - t0
                    nc.gpsimd.dma_gather(
                        out_ap=gwt[:, t0:t1, :], in_ap=w_full[:, :],
                        idxs_ap=gidx_t[:, (win * tpwb + t0) * 8:
                                       (win * tpwb + t1) * 8],
                        num_idxs=nt_ * 128, num_idxs_reg=nt_ * 128,
                        elem_size=D, queue_num=win % 4)
                gw_tiles.append(gwt)

            for chunk in range(16 if not done else 0):
                acc = wk.tile([128, 512], BF16, tag="acc", name="acc")
                for sub in range(8):
                    lp = psl.tile([128, 512], F32, tag="lp", name="lp")
                    nc.tensor.matmul(
                        lp[:], lhsT=csv_lhsT[:, sub * 128:(sub + 1) * 128],
                        rhs=wiki_T[:, chunk * 512:(chunk + 1) * 512],
                        start=True, stop=True)
                    col = sub * 16 + chunk
                    if sub == 0:
                        nc.scalar.activation(
                            out=acc[:], in_=lp[:], func=AF.Exp, scale=TEMP_INV,
                            bias=bias_m40[:, 0:1],
                            accum_out=rs_all[:, col:col + 1])
                    else:
                        ex = wk.tile([128, 512], BF16, tag="ex", name="ex")
                        nc.scalar.activation(
                            out=ex[:], in_=lp[:], func=AF.Exp, scale=TEMP_INV,
                            bias=bias_m40[:, 0:1],
                            accum_out=rs_all[:, col:col + 1])
                        nc.vector.tensor_tensor(out=acc[:], in0=acc[:], in1=ex[:],
                                                op=OP.add)
                cs_ps = psc.tile([1, 512], F32, tag="csps", name="cs_ps")
                nc.tensor.matmul(cs_ps[:], lhsT=ones_bf[:], rhs=acc[:],
                                 start=True, stop=True)
                nc.vector.tensor_copy(
                    out=colsum[0:1, chunk * 512:(chunk + 1) * 512], in_=cs_ps[:])

                # pair-term window (gathers land while early chunks run)
                if do_b and chunk >= 16 - NWIN:
                    win = chunk - (16 - NWIN)
                    kps = psb.tile([128, 128], F32, tag="kps", name="kps")
                    for t in range(tpwb):
                        nc.tensor.matmul(
                            kps[:], lhsT=ob_all[:, win * tpwb + t, :],
                            rhs=gw_tiles[win][:, t, :],
                            start=(t == 0), stop=(t == tpwb - 1))
                    kb = wk.tile([128, 128], BF16, tag="kb", name="kb")
                    nc.scalar.copy(out=kb[:], in_=kps[:])
                    scr = wk.tile([128, 128], F32, tag="scr", name="scr")
                    nc.vector.scalar_tensor_tensor(
                        out=scr[:], in0=kb[:], scalar=1.0, op0=OP.mult,
                        in1=cstrip[:, win, :], op1=OP.mult,
                        accum_out=u_acc[:, win:win + 1])

        # ================= final scalars =================
        # column-sum AllReduce (global over all C rows); trigger early so it
        # overlaps the v1/u reductions below.
        if not done:
            nc.sync.dma_start(out=cs_dram[:], in_=colsum[:])
            nc.gpsimd.collective_compute(
                "AllReduce", OP.add, replica_groups=[list(range(NCORES))],
                ins=[cs_dram.opt()], outs=[cs_ar.opt()])
            _finals(nc, tc, ctx, persist, cs_ar, u_acc, rs_all, g1s_t, g0f_t,
                    osc)

    nc.finalize()
    return nc


def _finals(nc, tc, ctx, persist, cs_ar, u_acc, rs_all, g1s_t, g0f_t, osc):
        fin = ctx.enter_context(tc.tile_pool(name="fin", bufs=1))
        out_sc = fin.tile([1, 8], F32)
        nc.vector.memset(out_sc[:], 0.0)

        # u partial
        ured = fin.tile([128, 1], F32)
        nc.vector.tensor_reduce(out=ured[:], in_=u_acc[:],
                                axis=mybir.AxisListType.XYZW, op=OP.add)
        usc = fin.tile([1, 1], F32)
        nc.gpsimd.tensor_reduce(out=usc[:], in_=ured[:],
                                axis=mybir.AxisListType.C, op=OP.add)
        nc.vector.tensor_copy(out=out_sc[0:1, 0:1], in_=usc[:])

        # v1 partial: strip row lse
        rsum = fin.tile([128, NWIN], F32)
        for s in range(NWIN):
            nc.vector.tensor_reduce(
                out=rsum[:, s:s + 1], in_=rs_all[:, s * 16:(s + 1) * 16],
                axis=mybir.AxisListType.XYZW, op=OP.add)
        _safe_ln(nc, fin, rsum, NWIN, "r")
        scr1 = fin.tile([128, NWIN], F32)
        v1p = fin.tile([128, 1], F32)
        nc.vector.scalar_tensor_tensor(
            out=scr1[:], in0=rsum[:], scalar=1.0, op0=OP.mult,
            in1=g1s_t[:], op1=OP.mult, accum_out=v1p[:])
        v1 = fin.tile([1, 1], F32)
        nc.gpsimd.tensor_reduce(out=v1[:], in_=v1p[:],
                                axis=mybir.AxisListType.C, op=OP.add)
        nc.vector.tensor_copy(out=out_sc[0:1, 1:2], in_=v1[:])

        # v0: global col lse (identical on every core after the AllReduce)
        cs_t = fin.tile([128, W // 128], F32)
        nc.sync.dma_start(
            out=cs_t[:], in_=cs_ar.rearrange("a (j p) -> p (a j)", p=128))
        _safe_ln(nc, fin, cs_t, W // 128, "c")
        scr0 = fin.tile([128, W // 128], F32)
        v0p = fin.tile([128, 1], F32)
        nc.vector.scalar_tensor_tensor(
            out=scr0[:], in0=cs_t[:], scalar=1.0, op0=OP.mult,
            in1=g0f_t[:], op1=OP.mult, accum_out=v0p[:])
        v0 = fin.tile([1, 1], F32)
        nc.gpsimd.tensor_reduce(out=v0[:], in_=v0p[:],
                                axis=mybir.AxisListType.C, op=OP.add)
        nc.vector.tensor_copy(out=out_sc[0:1, 2:3], in_=v0[:])

        nc.sync.dma_start(out=osc[:, :], in_=out_sc[:])


# ------------------------------------------------------------------- host ---

def _wrap16(a):
    """[num] int16 -> [128, num//16] gather-index layout (16-wrap, 8x repl)."""
    return np.ascontiguousarray(np.tile(a.reshape(-1, 16).T, (8, 1)))


def _window_pack(ids_sorted, payload_rows, values, tpw):
    """Pack sorted-by-id rows into 64 global windows x tpw tiles x 128 rows.

    ids_sorted: [M] sorted ids in [0, C); payload_rows: [M] row indices into
    the payload table (or -1); values: [M] f32 one-hot values.
    Returns (gi [nt128] payload row per slot (-1 pad), rel [nt128] within-
    window id (0 pad), val [nt128] one-hot value (0 pad)) where nt128 =
    64*tpw*128, window-major.
    """
    nwin_g = C // 128
    nt128 = nwin_g * tpw * 128
    gi = np.full(nt128, -1, np.int64)
    rel = np.zeros(nt128, np.int64)
    val = np.zeros(nt128, np.float32)
    starts = np.searchsorted(ids_sorted, np.arange(nwin_g) * 128)
    ends = np.searchsorted(ids_sorted, np.arange(1, nwin_g + 1) * 128)
    for w in range(nwin_g):
        s, e = starts[w], ends[w]
        cnt = e - s
        base = w * tpw * 128
        gi[base:base + cnt] = payload_rows[s:e]
        rel[base:base + cnt] = ids_sorted[s:e] - w * 128
        val[base:base + cnt] = values[s:e]
    return gi, rel, val


def _onehot_tiles(rel, val, nt):
    """[nt*128] rel/val -> [128, nt*128] bf16 one-hot input layout."""
    flat = np.zeros((nt * 128, 128), np.float32)
    rows = np.arange(nt * 128)
    m = val != 0.0
    flat[rows[m], rel[m]] = val[m]
    return np.ascontiguousarray(
        flat.astype(ml_dtypes.bfloat16).reshape(nt, 128, 128)
        .transpose(1, 0, 2).reshape(128, nt * 128))


def _payload_tiles(gi, table_bf16, nt):
    """Gather padded payload rows -> [128, nt*D] bf16 input layout."""
    out = np.zeros((nt * 128, D), ml_dtypes.bfloat16)
    m = gi >= 0
    out[m] = table_bf16[gi[m]]
    return np.ascontiguousarray(
        out.reshape(nt, 128, D).transpose(1, 0, 2).reshape(128, nt * D))


_CACHE = {}


def _run(inputs, trace=False, tmpdir=None):
    f1 = np.asarray(inputs["f1"], np.float32)
    f2 = np.asarray(inputs["f2"], np.float32)
    ci = np.asarray(inputs["csv_ids"]).astype(np.int64)
    wi = np.asarray(inputs["wiki_ids"]).astype(np.int64)

    cnt_c = np.bincount(ci, minlength=C)
    cnt_w = np.bincount(wi, minlength=W)
    r_c = (1.0 / np.maximum(cnt_c, 1)).astype(np.float32)
    r_w = (1.0 / np.maximum(cnt_w, 1)).astype(np.float32)
    g_c = (cnt_c > 0).astype(np.float32)
    g_w = (cnt_w > 0).astype(np.float32)

    # unique pairs (the reference's mask semantics)
    up = np.unique(ci * W + wi)
    uc = up // W
    uw = up % W
    ru = np.maximum(np.bincount(uc, minlength=C), 1).astype(np.float32)
    cu = np.maximum(np.bincount(uw, minlength=W), 1).astype(np.float32)
    s_pair = (1.0 / ru)[uc] + (1.0 / cu)[uw]      # already sorted by c

    # phase A routing: rows sorted by id, global windows of 128 ids
    permc = np.argsort(ci, kind="stable")
    permw = np.argsort(wi, kind="stable")
    sc = ci[permc]
    sw = wi[permw]

    wc_cnt = np.bincount(sc >> 7, minlength=C // 128)
    ww_cnt = np.bincount(sw >> 7, minlength=C // 128)
    ub_cnt = np.bincount(uc >> 7, minlength=C // 128)
    tpwa = int(max(-(-wc_cnt.max() // 128), -(-ww_cnt.max() // 128)))
    tpwb = int(-(-ub_cnt.max() // 128))

    import os as _os
    stop = _os.environ.get("KSTOP", "FULL")
    key = (tpwa, tpwb, stop)
    if key not in _CACHE:
        _CACHE[key] = _build(tpwa, tpwb, stop=stop)
    nc = _CACHE[key]

    nta = NWIN * tpwa
    ntb = NWIN * tpwb

    f1b = f1.astype(ml_dtypes.bfloat16)
    f2b = f2.astype(ml_dtypes.bfloat16)

    ones_c = np.ones(len(sc), np.float32)
    gi_c, rel_c, val_c = _window_pack(sc, permc, ones_c, tpwa)
    gi_w, rel_w, val_w = _window_pack(sw, permw, ones_c, tpwa)
    gi_b, rel_b, val_b = _window_pack(uc, uw, s_pair, tpwb)

    wpc = 64 // NCORES * tpwa * 128      # phase A slots per core
    wpb = 64 // NCORES * tpwb * 128
    g0_arr = np.ascontiguousarray(g_w.reshape(W // 128, 128).T)

    in_maps = []
    for i in range(NCORES):
        sa = slice(i * wpc, (i + 1) * wpc)
        sb = slice(i * wpb, (i + 1) * wpb)
        gidx_cols = []
        for w in range(NWIN):
            sw_ = slice(i * wpb + w * tpwb * 128, i * wpb + (w + 1) * tpwb * 128)
            idxs = gi_b[sw_].copy()
            idxs[idxs < 0] = 0
            gidx_cols.append(_wrap16(idxs.astype(np.int16)))
        g1s_arr = np.ascontiguousarray(
            g_c[i * STRIP:(i + 1) * STRIP].reshape(NWIN, 128).T)
        in_maps.append({
            "fw": _payload_tiles(gi_w[sa], f2b, nta),
            "ow": _onehot_tiles(rel_w[sa], val_w[sa], nta),
            "fc": _payload_tiles(gi_c[sa], f1b, nta),
            "oc": _onehot_tiles(rel_c[sa], val_c[sa], nta),
            "ob": _onehot_tiles(rel_b[sb], val_b[sb], ntb),
            "rwc": np.ascontiguousarray(
                r_c[i * STRIP:(i + 1) * STRIP].reshape(NWIN, 128).T),
            "rww": np.ascontiguousarray(
                r_w[i * STRIP:(i + 1) * STRIP].reshape(NWIN, 128).T),
            "gidx": np.hstack(gidx_cols),
            "g1s": g1s_arr, "g0f": g0_arr,
        })

    res = run_bass_kernel_spmd(nc, in_maps, core_ids=list(range(NCORES)),
                               trace=trace, tmpdir=tmpdir)

    # combine: per core osc = [u, v1_partial, v0]
    sc_out = np.stack([res.results[i]["osc"][0] for i in range(NCORES)])
    u = float(sc_out[:, 0].sum())
    v1 = float(sc_out[:, 1].sum())          # per-strip partials
    v0 = float(sc_out[0, 2])                # identical on every core
    G1 = float(g_c.sum())
    G0 = float(g_w.sum())
    loss = (0.5 / C) * ((v1 + SHIFT * G1) + (v0 + SHIFT * G0) - TEMP_INV * u)
    return np.float32(loss), res


def kernel(**inputs) -> np.ndarray:
    out, _ = _run(inputs)
    return out
